# revision 15
# baseline (speedup 1.0000x reference)
"""Trainium2 Bass kernel for nn_CGPCoupler (sparse Clebsch-Gordan bilinear coupling).

Reference computation:
    out[:, ro] += x1[:, r1] * x2[:, r2] * cg        (nnz = 9856 sparse entries)

The index triples come in 16-wide aligned runs, so the op factors over 16-element
"subslots" (40 of them in the 640-dim rep space):

    out_O  +=  c_t * (x1_A (*) x2_B)     for 616 (A,B,O) terms, 308 distinct (A,B)

v5 dataflow (PE-issue-count minimized; measured ~300-380ns per N=512 matmul is the
binding resource, so the kernel uses the minimum 10 matmul passes = 160 MMs):

    layout:  x2s strips [128/64, f]: products sorted by B give each 128-row product
             chunk a <=34-row source window, placed on a 32/64-aligned strip so the
             5 gather matmuls sit on distinct PE row-groups (LDWEIGHTS pull-ahead +
             partial row-tile concurrency)
    host:    x1g = x1 replicated into product-row order (numpy fancy-index),
             streamed from HBM in 524KB transfers (no on-chip gather for side 1)
    1. G2 = SEL^T @ x2s       (TensorE one-hot selection, K=6..34 per chunk)
    2. P  = x1g * G2          (G2 evacuated fp16 by ScalarE(4)/VectorE(1); multiply
                               split VectorE(3 chunks, 2x fp16)/GpSimd(2))
    3. out = W^T @ P          (TensorE, CG coeffs folded into fp16 W, PSUM-
                               accumulated over the 5 chunks; evacuated as fp16)

Host-side numpy work (layout shuffles, building SEL/W/x1g) is preprocessing of
inputs/constants; all arithmetic combining x1 and x2 happens on the NeuronCores.
"""

import os
import sys
import types

import numpy as np


def _ensure_ntff_hook():
    """concourse's trace path imports antenv.axon_hooks, which this image's
    antenv lacks. Provide it (and register the real profiling hook when the
    axon boot module is available) so tracing works instead of crashing."""
    try:
        import antenv
    except ImportError:
        return
    if getattr(antenv, "axon_hooks", None) is not None:
        return
    try:
        from antenv import axon_hooks  # noqa: F401
        return
    except ImportError:
        pass
    mod = types.ModuleType("antenv.axon_hooks")
    state = {"hook": None}
    mod.set_axon_ntff_profile_hook = lambda h: state.__setitem__("hook", h)
    mod.get_axon_ntff_profile_hook = lambda: state["hook"]
    sys.modules["antenv.axon_hooks"] = mod
    antenv.axon_hooks = mod
    try:
        from trn_agent_boot.trn_boot import _ntff_profile_via_ctypes
        so = "/opt/axon/libaxon_pjrt.so"
        if os.path.exists(so):
            mod.set_axon_ntff_profile_hook(_ntff_profile_via_ctypes(so))
    except Exception:
        pass


_ensure_ntff_hook()

N = 8192
DIM = 640
NCORES = 8
NLOC = N // NCORES          # rows per core
NSUB = DIM // 16            # 40 subslots
P_IN = NSUB * 2             # 80 half-rows: (subslot, ch-half)
CHH = 8                     # channels per half
FTOT = NLOC * CHH           # 8192 free elements per partition
FSUP = 1024                 # free-dim super-chunk
FCH = 512                   # free-dim chunk per matmul (one PSUM bank, fp32)
NCH = 5                     # product chunks (616 rows -> 5x128)
NQUART = 4                  # input DMA split along the free dim

# strip placement for each chunk's gather source window: (tile, base, K)
# tile 0 = x2sa [128 rows], tile 1 = x2sb [64 rows]
_STRIPS = [(0, 64, 32), (0, 96, 32), (1, 0, 32), (1, 32, 32), (0, 0, 64)]

LAST_RESULTS = None         # BassKernelResults of the most recent run

_plan_cache = {}
_program_cache = {}


def _build_plan(cg, r1, r2, ro):
    """Derive the chunked plan from the sparse index lists.

    Returns (A2, win, SEL, W) where
      A2   [NCH, 128] int: product row -> x1f source half-row
      win  [NCH] list of b-half-rows forming each chunk's x2 source window
      SEL  [NCH][K_c, 128] f16: one-hot gather weights (local window row -> col)
      W    [NCH, 128, P_IN] f16: scatter weights (coeff per output of each product)
    """
    key = (r1.tobytes(), r2.tobytes(), ro.tobytes(), cg.tobytes())
    hit = _plan_cache.get(key)
    if hit is not None:
        return hit

    A = r1 // 16
    B = r2 // 16
    O = ro // 16
    j = r1 % 16
    assert (r2 % 16 == j).all() and (ro % 16 == j).all(), \
        "index triples are not 16-aligned runs"
    assert A.max() < NSUB and B.max() < NSUB and O.max() < NSUB

    terms = {}   # (A,B,O) -> [coeff, covered-bitmask]
    for a, b, o, jj, c in zip(A.tolist(), B.tolist(), O.tolist(),
                              j.tolist(), cg.tolist()):
        k = (a, b, o)
        e = terms.get(k)
        if e is None:
            terms[k] = [c, 1 << jj]
        else:
            assert e[0] == c, "coefficient varies within a 16-run"
            assert not (e[1] >> jj) & 1, "duplicate (A,B,O,j) entry"
            e[1] |= 1 << jj
    for k, (c, mask) in terms.items():
        assert mask == 0xFFFF, f"term {k} covers only mask {mask:#x}"

    # sort products by (b, a): 128-row chunks then have small b-windows
    prods = sorted({(a, b) for (a, b, o) in terms},
                   key=lambda ab: (ab[1], ab[0]))
    assert len(prods) <= NCH * 64

    outs = {}    # (A,B) -> [(O, c), ...]
    for (a, b, o), (c, _) in terms.items():
        outs.setdefault((a, b), []).append((o, c))

    A2 = np.zeros((NCH, 128), np.int64)
    win, SEL = [], []
    W = np.zeros((NCH, 128, P_IN), np.float16)
    for cix in range(NCH):
        chunk = prods[cix * 64:(cix + 1) * 64]
        wrows = sorted({2 * b + hh for (a, b) in chunk for hh in (0, 1)})
        K = _STRIPS[cix][2]
        assert len(wrows) <= K, f"chunk {cix} window {len(wrows)} > {K}"
        rank = {r: i for i, r in enumerate(wrows)}
        sel = np.zeros((K, 128), np.float16)
        for d, (a, b) in enumerate(chunk):
            for hh in (0, 1):
                row = 2 * d + hh
                A2[cix, row] = 2 * a + hh
                sel[rank[2 * b + hh], row] = 1.0
                for o, c in outs[(a, b)]:
                    W[cix, row, 2 * o + hh] = c
        win.append(wrows)
        SEL.append(sel)

    out = (A2, win, SEL, W)
    _plan_cache[key] = out
    return out


def _pack_x(x):
    """[NLOC, 640] -> [80, NLOC*8] fp16: row p = subslot*2 + half, col = n*8 + ch."""
    return np.ascontiguousarray(
        x.reshape(NLOC, NSUB, 2, CHH).transpose(1, 2, 0, 3).reshape(P_IN, FTOT),
        dtype=np.float16)


def _unpack_out(o):
    """[80, NLOC*8] -> [NLOC, 640]."""
    return o.reshape(NSUB, 2, NLOC, CHH).transpose(2, 0, 1, 3).reshape(NLOC, DIM)


def _build_x2s(x2f, win):
    """Strip-packed gather sources: x2sa [128, FTOT], x2sb [64, FTOT]."""
    x2sa = np.zeros((128, FTOT), np.float16)
    x2sb = np.zeros((64, FTOT), np.float16)
    tiles = (x2sa, x2sb)
    for cix in range(NCH):
        t, base, K = _STRIPS[cix]
        for i, r in enumerate(win[cix]):
            tiles[t][base + i] = x2f[r]
    return x2sa, x2sb


def _build_program(nws):
    """v5: minimum-pass dataflow (see module docstring). nws = per-chunk
    used source-window row counts (contraction covers only DMA'd rows)."""
    import concourse.mybir as mybir
    import concourse.tile as tile
    from concourse import bacc
    from concourse.bass import ds, ts

    f32 = mybir.dt.float32
    f16 = mybir.dt.float16
    nc = bacc.Bacc("TRN2", target_bir_lowering=False)

    NSUP = FTOT // FSUP     # 8
    NJ = FSUP // FCH        # 2 matmul FD chunks per super-chunk
    FQ = FTOT // NQUART     # free-dim quarter per input dma
    SPQ = NSUP // NQUART    # supers per quarter

    x1gd = nc.dram_tensor("x1g", [NCH, 128, FTOT], f16, kind="ExternalInput")
    x2sad = nc.dram_tensor("x2sa", [128, FTOT], f16, kind="ExternalInput")
    x2sbd = nc.dram_tensor("x2sb", [64, FTOT], f16, kind="ExternalInput")
    seld = nc.dram_tensor("sel", [128, NCH * 128], f16, kind="ExternalInput")
    wd = nc.dram_tensor("wmat", [128, NCH * P_IN], f16, kind="ExternalInput")
    outd = nc.dram_tensor("outf", [P_IN, FTOT], f16, kind="ExternalOutput")

    # window row ranges to DMA per x2s tile: (tile, lo, hi)
    spans = [(0, 0, 34), (0, 64, 70), (0, 96, 110), (1, 0, 14), (1, 32, 50)]

    with tile.TileContext(nc) as tc:
        with tc.tile_pool(name="const", bufs=1) as constp, \
             tc.tile_pool(name="gsb", bufs=6) as gsb, \
             tc.tile_pool(name="psb", bufs=2 * NCH) as psb, \
             tc.tile_pool(name="og", bufs=4) as og, \
             tc.tile_pool(name="psg", bufs=3, space="PSUM") as psg, \
             tc.tile_pool(name="pso", bufs=2, space="PSUM") as pso:

            sel = constp.tile([128, NCH * 128], f16, tag="sel")
            nc.sync.dma_start(out=sel, in_=seld[:])
            w = constp.tile([128, NCH * P_IN], f16, tag="w")
            nc.sync.dma_start(out=w, in_=wd[:])

            x2q = {}     # (tile_idx, quarter) -> sbuf tile
            x1q = {}     # (chunk, quarter) -> sbuf tile
            for q in range(NQUART):
                qs = ds(q * FQ, FQ)
                for ti, dram, rows in ((0, x2sad, 128), (1, x2sbd, 64)):
                    t = constp.tile([rows, FQ], f16, tag=f"x2s{ti}q{q}")
                    for (tt, lo, hi) in spans:
                        if tt == ti:
                            nc.sync.dma_start(out=t[ds(lo, hi - lo)],
                                              in_=dram[ds(lo, hi - lo), qs])
                    x2q[(ti, q)] = t
                for sp in range(SPQ):
                    sup = q * SPQ + sp
                    for c in range(NCH):
                        t = constp.tile([128, FSUP], f16, tag=f"x1gc{c}s{sup}")
                        nc.sync.dma_start(
                            out=t, in_=x1gd[c, :, ds(sup * FSUP, FSUP)])
                        x1q[(c, sup)] = t

            SORDER = [0, 1, 2, 3, 4]
            def scatter(sup, pts, outps):
                for jj in range(NJ):
                    for i, c in enumerate(SORDER):
                        nc.tensor.matmul(outps[jj], w[:, ts(c, P_IN)],
                                         pts[c][:, ts(jj, FCH)],
                                         start=(i == 0), stop=(i == NCH - 1),
                                         skip_group_check=True)
                outt = og.tile([P_IN, FSUP], f16, tag="outt")
                nc.scalar.copy(out=outt[:, ts(0, FCH)], in_=outps[0])
                nc.scalar.copy(out=outt[:, ts(1, FCH)], in_=outps[1])
                nc.gpsimd.dma_start(out=outd[:, ds(sup * FSUP, FSUP)], in_=outt)

            # chunk issue order: c4 (row groups 0-1) first, then c0..c3 on
            # strips 2,3,0,1 - consecutive gathers sit on disjoint row groups.
            # c4/c0: V multiplies straight from PSUM (no evacuation; 1x but
            # saves the evac pass); c1/c2: S evac + V 2x mult; c3: S evac +
            # GpSimd mult (scatter consumes it last, so its latency hides)
            ORDER = [4, 0, 1, 2, 3]
            PSUM_MULT = {4, 0}
            MULT_G = {3}

            prev = None
            for sup in range(NSUP):
                q, so = sup // SPQ, (sup % SPQ) * FSUP
                ssl = ds(so, FSUP)

                pts = [None] * NCH
                for c in ORDER:
                    ti, base, K = _STRIPS[c]
                    nw = nws[c]
                    xt = x2q[(ti, q)]
                    gp = psg.tile([128, FSUP], f32, tag="gp")
                    for jj in range(NJ):
                        # bass auto-derives tile_position but rejects base 96
                        nc.tensor.matmul(
                            gp[:, ts(jj, FCH)],
                            sel[ds(base, nw), ts(c, 128)],
                            xt[ds(base, nw), so + jj * FCH:so + (jj + 1) * FCH],
                            start=True, stop=True,
                            tile_position=(base, 0))
                    pt = psb.tile([128, FSUP], f16, tag="pt")
                    if c in PSUM_MULT:
                        nc.vector.tensor_mul(pt, x1q[(c, sup)], gp)
                    else:
                        g2s = gsb.tile([128, FSUP], f16, tag="g2s")
                        nc.scalar.copy(out=g2s, in_=gp)
                        if c in MULT_G:
                            nc.gpsimd.tensor_mul(pt, x1q[(c, sup)], g2s)
                        else:
                            nc.vector.tensor_mul(pt, x1q[(c, sup)], g2s)
                    pts[c] = pt

                if prev is not None:
                    scatter(*prev)
                outps = [pso.tile([P_IN, FCH], f32, tag="outp",
                                  name=f"outp{sup}_{jj}")
                         for jj in range(NJ)]
                prev = (sup, pts, outps)

            scatter(*prev)

    nc.compile()
    return nc


def kernel(x1, x2, cg_tilde, repids_in1, repids_in2, repids_out, out_dim=DIM,
           **_ignored):
    global LAST_RESULTS
    import concourse.bass_utils as _bu
    from concourse.bass_utils import run_bass_kernel_spmd
    # the trace path uploads artifacts to S3, which this container can't reach
    if not getattr(_bu.upload_artifacts, "_local", False):
        _bu.upload_artifacts = lambda tmpdir: "local://" + tmpdir
        _bu.upload_artifacts._local = True

    x1 = np.ascontiguousarray(np.asarray(x1), dtype=np.float32)
    x2 = np.ascontiguousarray(np.asarray(x2), dtype=np.float32)
    cg = np.asarray(cg_tilde, dtype=np.float32)
    r1 = np.asarray(repids_in1, dtype=np.int64)
    r2 = np.asarray(repids_in2, dtype=np.int64)
    ro = np.asarray(repids_out, dtype=np.int64)
    out_dim = int(out_dim)
    assert x1.shape == (N, DIM) and x2.shape == (N, DIM) and out_dim == DIM

    A2, win, SEL, W = _build_plan(cg, r1, r2, ro)

    nws = tuple(len(w) for w in win)
    nc = _program_cache.get(nws)
    if nc is None:
        nc = _build_program(nws)
        _program_cache[nws] = nc

    selp = np.zeros((128, NCH * 128), np.float16)
    for c in range(NCH):
        _, base, K = _STRIPS[c]
        selp[base:base + K, c * 128:(c + 1) * 128] = SEL[c]
    wp = np.zeros((128, NCH * P_IN), np.float16)
    for c in range(NCH):
        wp[:, c * P_IN:(c + 1) * P_IN] = W[c]

    in_maps = []
    for cr in range(NCORES):
        sl = slice(cr * NLOC, (cr + 1) * NLOC)
        x1f = _pack_x(x1[sl])
        x2f = _pack_x(x2[sl])
        x2sa, x2sb = _build_x2s(x2f, win)
        in_maps.append({
            "x1g": np.ascontiguousarray(x1f[A2]),
            "x2sa": x2sa, "x2sb": x2sb,
            "sel": selp, "wmat": wp,
        })

    res = run_bass_kernel_spmd(nc, in_maps, core_ids=list(range(NCORES)))
    LAST_RESULTS = res

    out = np.empty((N, DIM), np.float32)
    for cr in range(NCORES):
        out[cr * NLOC:(cr + 1) * NLOC] = _unpack_out(
            np.asarray(res.results[cr]["outf"], dtype=np.float32))
    return out


def _numpy_model(x1, x2, cg, r1, r2, ro):
    """Host-side model of the device dataflow (including fp16 quantization),
    for validating index logic and predicting the on-device error."""
    A2, win, SEL, W = _build_plan(cg, r1, r2, ro)
    out = np.empty_like(x1)
    for cr in range(NCORES):
        sl = slice(cr * NLOC, (cr + 1) * NLOC)
        x1f = _pack_x(x1[sl])
        x2f = _pack_x(x2[sl])
        x2sa, x2sb = _build_x2s(x2f, win)
        tiles = (x2sa, x2sb)
        x1g = x1f[A2]
        outf = np.zeros((P_IN, FTOT), np.float32)
        for c in range(NCH):
            ti, base, K = _STRIPS[c]
            src = tiles[ti][base:base + K].astype(np.float32)
            g2 = (SEL[c].astype(np.float32).T @ src).astype(np.float16)
            pt = (x1g[c].astype(np.float32) * g2.astype(np.float32)
                  ).astype(np.float16).astype(np.float32)
            outf += W[c].astype(np.float32).T @ pt
        out[sl] = _unpack_out(outf.astype(np.float16).astype(np.float32))
    return out


# revision 17
# speedup vs baseline: 1.1748x; 1.1748x over previous
"""Trainium2 Bass kernel for nn_CGPCoupler (sparse Clebsch-Gordan bilinear coupling).

Reference computation:
    out[:, ro] += x1[:, r1] * x2[:, r2] * cg        (nnz = 9856 sparse entries)

Structure exploited: the index triples come in 16-wide aligned runs, so the whole
op factors over 16-element "subslots" (40 of them in the 640-dim rep space):

    out_O  +=  c_t * (x1_A  (*)  x2_B)      for 616 subslot-triples t=(A,B,O,c)

with only D=308 distinct (A,B) products. Dataflow (per core, data parallel over
the batch dim, 1024 rows/core, fp16 datapath / fp32 PSUM):

    layout:  x2f[p = subslot*2 + ch_half (80 partitions), f = n*8 + ch_lo (8192)]
    host:    x1g = x1 replicated into product-row order (numpy fancy-index),
             streamed straight from HBM (no on-chip gather for side 1)
    1. G2 = SEL2^T @ x2f      (TensorE one-hot selection matmul -> PSUM)
    2. P  = x1g * G2          (VectorE; 4 of 5 chunks evacuated to SBUF fp16 by
                               ScalarE first so the multiply runs in 2x mode)
    3. out = W^T @ P          (TensorE, CG coeffs folded into constant fp16 W,
                               PSUM-accumulated over the 5 product-row chunks)

Host-side numpy work (layout shuffles, building SEL2/W/x1g) is preprocessing of
inputs/constants; all arithmetic combining x1 and x2 happens on the NeuronCores.
"""

import os
import sys
import types

import numpy as np


def _ensure_ntff_hook():
    """concourse's trace path imports antenv.axon_hooks, which this image's
    antenv lacks. Provide it (and register the real profiling hook when the
    axon boot module is available) so tracing works instead of crashing."""
    try:
        import antenv
    except ImportError:
        return
    if getattr(antenv, "axon_hooks", None) is not None:
        return
    try:
        from antenv import axon_hooks  # noqa: F401
        return
    except ImportError:
        pass
    mod = types.ModuleType("antenv.axon_hooks")
    state = {"hook": None}
    mod.set_axon_ntff_profile_hook = lambda h: state.__setitem__("hook", h)
    mod.get_axon_ntff_profile_hook = lambda: state["hook"]
    sys.modules["antenv.axon_hooks"] = mod
    antenv.axon_hooks = mod
    try:
        from trn_agent_boot.trn_boot import _ntff_profile_via_ctypes
        so = "/opt/axon/libaxon_pjrt.so"
        if os.path.exists(so):
            mod.set_axon_ntff_profile_hook(_ntff_profile_via_ctypes(so))
    except Exception:
        pass


_ensure_ntff_hook()

N = 8192
DIM = 640
NCORES = 8
NLOC = N // NCORES          # rows per core
NSUB = DIM // 16            # 40 subslots
P_IN = NSUB * 2             # 80 partitions: (subslot, ch-half)
CHH = 8                     # channels per half
FTOT = NLOC * CHH           # 8192 free elements per partition
FSUP = 2048                 # free-dim super-chunk (per DMA / out tile)
FCH = 512                   # free-dim chunk per matmul (one PSUM bank, fp32)

LAST_RESULTS = None         # BassKernelResults of the most recent run

_matrices_cache = {}
_program_cache = {}


def _build_matrices(cg, r1, r2, ro):
    """Derive subslot terms from the sparse index lists and build the constant
    SEL1/SEL2/W matrices. Everything is validated with asserts."""
    key = (r1.tobytes(), r2.tobytes(), ro.tobytes(), cg.tobytes())
    hit = _matrices_cache.get(key)
    if hit is not None:
        return hit

    A = r1 // 16
    B = r2 // 16
    O = ro // 16
    j = r1 % 16
    assert (r2 % 16 == j).all() and (ro % 16 == j).all(), \
        "index triples are not 16-aligned runs"
    assert A.max() < NSUB and B.max() < NSUB and O.max() < NSUB

    terms = {}   # (A,B,O) -> [coeff, covered-bitmask]
    for a, b, o, jj, c in zip(A.tolist(), B.tolist(), O.tolist(),
                              j.tolist(), cg.tolist()):
        k = (a, b, o)
        e = terms.get(k)
        if e is None:
            terms[k] = [c, 1 << jj]
        else:
            assert e[0] == c, "coefficient varies within a 16-run"
            assert not (e[1] >> jj) & 1, "duplicate (A,B,O,j) entry"
            e[1] |= 1 << jj
    for k, (c, mask) in terms.items():
        assert mask == 0xFFFF, f"term {k} covers only mask {mask:#x}"

    products = sorted({(a, b) for (a, b, o) in terms})
    pidx = {ab: d for d, ab in enumerate(products)}
    D = len(products)
    D2 = 2 * D
    nchunks = (D2 + 127) // 128
    D2p = 128 * nchunks

    SEL2 = np.zeros((P_IN, D2p), np.float16)
    A2 = np.zeros(D2p, np.int64)   # product row -> source row in x1f layout
    W = np.zeros((D2p, P_IN), np.float16)
    for (a, b), d in pidx.items():
        for hh in (0, 1):
            SEL2[b * 2 + hh, 2 * d + hh] = 1.0
            A2[2 * d + hh] = a * 2 + hh
    for (a, b, o), (c, _) in terms.items():
        d = pidx[(a, b)]
        for hh in (0, 1):
            W[2 * d + hh, o * 2 + hh] = c

    # pack W row-chunks side by side: WPACK[:, c*P_IN:(c+1)*P_IN] = W[c*128:...]
    WPACK = np.zeros((128, nchunks * P_IN), np.float16)
    for c in range(nchunks):
        WPACK[:, c * P_IN:(c + 1) * P_IN] = W[c * 128:(c + 1) * 128, :]

    out = (A2, SEL2, WPACK, nchunks)
    _matrices_cache[key] = out
    return out


def _pack_x(x):
    """[NLOC, 640] -> [80, NLOC*8] fp16: row p = subslot*2 + half, col = n*8 + ch."""
    return np.ascontiguousarray(
        x.reshape(NLOC, NSUB, 2, CHH).transpose(1, 2, 0, 3).reshape(P_IN, FTOT),
        dtype=np.float16)


def _unpack_out(o):
    """[80, NLOC*8] -> [NLOC, 640]."""
    return o.reshape(NSUB, 2, NLOC, CHH).transpose(2, 0, 1, 3).reshape(NLOC, DIM)


def _build_program(nchunks):
    """fp16 datapath, v3: the G1 side (x1 replicated into product-row order) is
    prepared on the host and streamed straight from HBM — no gather matmul and
    no PSUM round-trip for it. On-chip work per super-chunk of 1024 free elems:
      - G2 = SEL2^T @ x2f  (TensorE -> PSUM)
      - P[c] = x1g[c] * G2[c]   (VectorE; for NEVAC chunks ScalarE first
        evacuates G2 to SBUF fp16 so the multiply runs in 2x 16-bit mode)
      - out += W[c]^T @ P[c]    (TensorE, PSUM-accumulated)
    """
    import concourse.mybir as mybir
    import concourse.tile as tile
    from concourse import bacc
    from concourse.bass import ds, ts

    f32 = mybir.dt.float32
    f16 = mybir.dt.float16
    nc = bacc.Bacc("TRN2", target_bir_lowering=False)

    FSUP_ = 1024            # free-dim super-chunk
    NSUP = FTOT // FSUP_    # 8
    NJ = FSUP_ // FCH       # 2 matmul FD chunks per super-chunk
    NEVAC = 3               # chunks whose G2 is evacuated by ScalarE

    x1gd = nc.dram_tensor("x1g", [nchunks, 128, FTOT], f16, kind="ExternalInput")
    x2d = nc.dram_tensor("x2f", [P_IN, FTOT], f16, kind="ExternalInput")
    s2d = nc.dram_tensor("sel2", [P_IN, nchunks * 128], f16, kind="ExternalInput")
    wd = nc.dram_tensor("wmat", [128, nchunks * P_IN], f16, kind="ExternalInput")
    outd = nc.dram_tensor("outf", [P_IN, FTOT], f16, kind="ExternalOutput")

    with tile.TileContext(nc) as tc:
        with tc.tile_pool(name="const", bufs=1) as constp, \
             tc.tile_pool(name="x1io", bufs=1) as x1io, \
             tc.tile_pool(name="x2io", bufs=3) as x2io, \
             tc.tile_pool(name="gsb", bufs=4) as gsb, \
             tc.tile_pool(name="psb", bufs=2 * nchunks) as psb, \
             tc.tile_pool(name="og", bufs=3) as og, \
             tc.tile_pool(name="psg", bufs=3, space="PSUM") as psg, \
             tc.tile_pool(name="pso", bufs=2, space="PSUM") as pso:

            s2 = constp.tile([P_IN, nchunks * 128], f16, tag="s2")
            nc.scalar.dma_start(out=s2, in_=s2d[:])
            w = constp.tile([128, nchunks * P_IN], f16, tag="w")
            nc.scalar.dma_start(out=w, in_=wd[:])

            FQ = 2 * FSUP_
            x1q = {}
            for q in range(NSUP // 2):
                for c in range(nchunks):
                    t = x1io.tile([128, FQ], f16, tag=f"x1gc{c}q{q}",
                                  name=f"x1g_{c}_{q}")
                    # the big streaming input gets its own (sync) HWDGE queue
                    nc.sync.dma_start(out=t, in_=x1gd[c, :, ds(q * FQ, FQ)])
                    x1q[(c, q)] = t

            for sup in range(NSUP):
                ssl = ds(sup * FSUP_, FSUP_)
                hsl = ds((sup % 2) * FSUP_, FSUP_)
                x2t = x2io.tile([P_IN, FSUP_], f16, tag="x2t")
                # SWDGE (GpSimd) queue: keeps ScalarE free for evacuations
                nc.gpsimd.dma_start(out=x2t, in_=x2d[:, ssl])
                x1gt = [x1q[(c, sup // 2)][:, hsl] for c in range(nchunks)]

                pts = []
                for c in range(nchunks):
                    g2p = psg.tile([128, FSUP_], f32, tag="gp")
                    for j in range(NJ):
                        nc.tensor.matmul(g2p[:, ts(j, FCH)], s2[:, ts(c, 128)],
                                         x2t[:, ts(j, FCH)], start=True, stop=True)
                    pt = psb.tile([128, FSUP_], f16, tag="pt")
                    if c < 2:
                        # PSUM-direct multiply on V (1x, but skips the evac)
                        nc.vector.tensor_mul(pt, x1gt[c], g2p)
                    else:
                        g2s = gsb.tile([128, FSUP_], f16, tag="g2s")
                        nc.scalar.copy(out=g2s, in_=g2p)
                        if c == 2:
                            # GpSimd offload; scatter consumes c2 mid-chain a
                            # full super later, so its ~2.3us latency hides
                            nc.gpsimd.tensor_mul(pt, x1gt[c], g2s)
                        else:
                            nc.vector.tensor_mul(pt, x1gt[c], g2s)
                    pts.append(pt)

                # scatter: W[c] PSUM-accumulated over c, one bank per j
                outps = []
                for j in range(NJ):
                    outp_j = pso.tile([P_IN, FCH], f32, tag="outp")
                    outps.append(outp_j)
                for c in range(nchunks):
                    for j in range(NJ):
                        nc.tensor.matmul(outps[j], w[:, ts(c, P_IN)],
                                         pts[c][:, ts(j, FCH)],
                                         start=(c == 0), stop=(c == nchunks - 1),
                                         skip_group_check=True)
                outt = og.tile([P_IN, FSUP_], f16, tag="outt")
                if sup == NSUP - 1:
                    # final super-chunk is the kernel tail: parallelize the two
                    # copies across V/S and ship via the low-latency HWDGE path
                    nc.vector.tensor_copy(out=outt[:, ts(0, FCH)], in_=outps[0])
                    nc.scalar.copy(out=outt[:, ts(1, FCH)], in_=outps[1])
                    nc.scalar.dma_start(out=outd[:, ssl], in_=outt)
                else:
                    for j in range(NJ):
                        nc.vector.tensor_copy(out=outt[:, ts(j, FCH)], in_=outps[j])
                    nc.gpsimd.dma_start(out=outd[:, ssl], in_=outt)
    nc.compile()
    return nc


def kernel(x1, x2, cg_tilde, repids_in1, repids_in2, repids_out, out_dim=DIM,
           **_ignored):
    global LAST_RESULTS
    import concourse.bass_utils as _bu
    from concourse.bass_utils import run_bass_kernel_spmd
    # the trace path uploads artifacts to S3, which this container can't reach
    if not getattr(_bu.upload_artifacts, "_local", False):
        _bu.upload_artifacts = lambda tmpdir: "local://" + tmpdir
        _bu.upload_artifacts._local = True

    x1 = np.ascontiguousarray(np.asarray(x1), dtype=np.float32)
    x2 = np.ascontiguousarray(np.asarray(x2), dtype=np.float32)
    cg = np.asarray(cg_tilde, dtype=np.float32)
    r1 = np.asarray(repids_in1, dtype=np.int64)
    r2 = np.asarray(repids_in2, dtype=np.int64)
    ro = np.asarray(repids_out, dtype=np.int64)
    out_dim = int(out_dim)
    assert x1.shape == (N, DIM) and x2.shape == (N, DIM) and out_dim == DIM

    A2, SEL2, WPACK, nchunks = _build_matrices(cg, r1, r2, ro)

    nc = _program_cache.get(nchunks)
    if nc is None:
        nc = _build_program(nchunks)
        _program_cache[nchunks] = nc

    in_maps = []
    for c in range(NCORES):
        sl = slice(c * NLOC, (c + 1) * NLOC)
        x1f = _pack_x(x1[sl])
        in_maps.append({
            "x1g": np.ascontiguousarray(
                x1f[A2].reshape(nchunks, 128, FTOT)),
            "x2f": _pack_x(x2[sl]),
            "sel2": SEL2,
            "wmat": WPACK,
        })

    res = run_bass_kernel_spmd(nc, in_maps, core_ids=list(range(NCORES)))
    LAST_RESULTS = res

    out = np.empty((N, DIM), np.float32)
    for c in range(NCORES):
        out[c * NLOC:(c + 1) * NLOC] = _unpack_out(
            np.asarray(res.results[c]["outf"], dtype=np.float32))
    return out


def _numpy_model(x1, x2, cg, r1, r2, ro):
    """Host-side model of the device dataflow (including fp16 quantization),
    for validating index logic and predicting the on-device error."""
    A2, SEL2, WPACK, nchunks = _build_matrices(cg, r1, r2, ro)
    W = np.zeros((128 * nchunks, P_IN), np.float32)
    for c in range(nchunks):
        W[c * 128:(c + 1) * 128, :] = WPACK[:, c * P_IN:(c + 1) * P_IN].astype(
            np.float32)
    out = np.empty_like(x1)
    for c in range(NCORES):
        sl = slice(c * NLOC, (c + 1) * NLOC)
        x1f = _pack_x(x1[sl])
        x2f = _pack_x(x2[sl]).astype(np.float32)
        g1 = x1f[A2].astype(np.float32)
        g2 = (SEL2.astype(np.float32).T @ x2f).astype(np.float16)  # worst branch
        p = (g1 * g2.astype(np.float32)).astype(np.float16)
        outf = W.T @ p.astype(np.float32)
        out[sl] = _unpack_out(outf)
    return out



# revision 18
# speedup vs baseline: 1.2668x; 1.0783x over previous
"""Trainium2 Bass kernel for nn_CGPCoupler (sparse Clebsch-Gordan bilinear coupling).

Reference computation:
    out[:, ro] += x1[:, r1] * x2[:, r2] * cg        (nnz = 9856 sparse entries)

Structure exploited: the index triples come in 16-wide aligned runs, so the whole
op factors over 16-element "subslots" (40 of them in the 640-dim rep space):

    out_O  +=  c_t * (x1_A  (*)  x2_B)      for 616 subslot-triples t=(A,B,O,c)

with only D=308 distinct (A,B) products. Dataflow (per core, data parallel over
the batch dim, 1024 rows/core, fp16 datapath / fp32 PSUM):

    layout:  x2f[p = subslot*2 + ch_half (80 partitions), f = n*8 + ch_lo (8192)]
    host:    x1g = x1 replicated into product-row order (numpy fancy-index),
             streamed straight from HBM (no on-chip gather for side 1)
    1. G2 = SEL2^T @ x2f      (TensorE one-hot selection matmul -> PSUM)
    2. P  = x1g * G2          (VectorE; 4 of 5 chunks evacuated to SBUF fp16 by
                               ScalarE first so the multiply runs in 2x mode)
    3. out = W^T @ P          (TensorE, CG coeffs folded into constant fp16 W,
                               PSUM-accumulated over the 5 product-row chunks)

Host-side numpy work (layout shuffles, building SEL2/W/x1g) is preprocessing of
inputs/constants; all arithmetic combining x1 and x2 happens on the NeuronCores.
"""

import os
import sys
import types

import numpy as np


def _ensure_ntff_hook():
    """concourse's trace path imports antenv.axon_hooks, which this image's
    antenv lacks. Provide it (and register the real profiling hook when the
    axon boot module is available) so tracing works instead of crashing."""
    try:
        import antenv
    except ImportError:
        return
    if getattr(antenv, "axon_hooks", None) is not None:
        return
    try:
        from antenv import axon_hooks  # noqa: F401
        return
    except ImportError:
        pass
    mod = types.ModuleType("antenv.axon_hooks")
    state = {"hook": None}
    mod.set_axon_ntff_profile_hook = lambda h: state.__setitem__("hook", h)
    mod.get_axon_ntff_profile_hook = lambda: state["hook"]
    sys.modules["antenv.axon_hooks"] = mod
    antenv.axon_hooks = mod
    try:
        from trn_agent_boot.trn_boot import _ntff_profile_via_ctypes
        so = "/opt/axon/libaxon_pjrt.so"
        if os.path.exists(so):
            mod.set_axon_ntff_profile_hook(_ntff_profile_via_ctypes(so))
    except Exception:
        pass


_ensure_ntff_hook()

N = 8192
DIM = 640
NCORES = 8
NLOC = N // NCORES          # rows per core
NSUB = DIM // 16            # 40 subslots
P_IN = NSUB * 2             # 80 partitions: (subslot, ch-half)
CHH = 8                     # channels per half
FTOT = NLOC * CHH           # 8192 free elements per partition
FSUP = 2048                 # free-dim super-chunk (per DMA / out tile)
FCH = 512                   # free-dim chunk per matmul (one PSUM bank, fp32)

LAST_RESULTS = None         # BassKernelResults of the most recent run

_matrices_cache = {}
_program_cache = {}


def _build_matrices(cg, r1, r2, ro):
    """Derive subslot terms from the sparse index lists and build the constant
    SEL1/SEL2/W matrices. Everything is validated with asserts."""
    key = (r1.tobytes(), r2.tobytes(), ro.tobytes(), cg.tobytes())
    hit = _matrices_cache.get(key)
    if hit is not None:
        return hit

    A = r1 // 16
    B = r2 // 16
    O = ro // 16
    j = r1 % 16
    assert (r2 % 16 == j).all() and (ro % 16 == j).all(), \
        "index triples are not 16-aligned runs"
    assert A.max() < NSUB and B.max() < NSUB and O.max() < NSUB

    terms = {}   # (A,B,O) -> [coeff, covered-bitmask]
    for a, b, o, jj, c in zip(A.tolist(), B.tolist(), O.tolist(),
                              j.tolist(), cg.tolist()):
        k = (a, b, o)
        e = terms.get(k)
        if e is None:
            terms[k] = [c, 1 << jj]
        else:
            assert e[0] == c, "coefficient varies within a 16-run"
            assert not (e[1] >> jj) & 1, "duplicate (A,B,O,j) entry"
            e[1] |= 1 << jj
    for k, (c, mask) in terms.items():
        assert mask == 0xFFFF, f"term {k} covers only mask {mask:#x}"

    products = sorted({(a, b) for (a, b, o) in terms})
    pidx = {ab: d for d, ab in enumerate(products)}
    D = len(products)
    D2 = 2 * D
    nchunks = (D2 + 127) // 128
    D2p = 128 * nchunks

    SEL2 = np.zeros((P_IN, D2p), np.float16)
    A2 = np.zeros(D2p, np.int64)   # product row -> source row in x1f layout
    W = np.zeros((D2p, P_IN), np.float16)
    for (a, b), d in pidx.items():
        for hh in (0, 1):
            SEL2[b * 2 + hh, 2 * d + hh] = 1.0
            A2[2 * d + hh] = a * 2 + hh
    for (a, b, o), (c, _) in terms.items():
        d = pidx[(a, b)]
        for hh in (0, 1):
            W[2 * d + hh, o * 2 + hh] = c

    # pack W row-chunks side by side: WPACK[:, c*P_IN:(c+1)*P_IN] = W[c*128:...]
    WPACK = np.zeros((128, nchunks * P_IN), np.float16)
    for c in range(nchunks):
        WPACK[:, c * P_IN:(c + 1) * P_IN] = W[c * 128:(c + 1) * 128, :]

    out = (A2, SEL2, WPACK, nchunks)
    _matrices_cache[key] = out
    return out


def _pack_x(x):
    """[NLOC, 640] -> [80, NLOC*8] fp16: row p = subslot*2 + half, col = n*8 + ch."""
    return np.ascontiguousarray(
        x.reshape(NLOC, NSUB, 2, CHH).transpose(1, 2, 0, 3).reshape(P_IN, FTOT),
        dtype=np.float16)


def _unpack_out(o):
    """[80, NLOC*8] -> [NLOC, 640]."""
    return o.reshape(NSUB, 2, NLOC, CHH).transpose(2, 0, 1, 3).reshape(NLOC, DIM)


def _build_program(nchunks):
    """fp16 datapath, v3: the G1 side (x1 replicated into product-row order) is
    prepared on the host and streamed straight from HBM — no gather matmul and
    no PSUM round-trip for it. On-chip work per super-chunk of 1024 free elems:
      - G2 = SEL2^T @ x2f  (TensorE -> PSUM)
      - P[c] = x1g[c] * G2[c]   (VectorE; for NEVAC chunks ScalarE first
        evacuates G2 to SBUF fp16 so the multiply runs in 2x 16-bit mode)
      - out += W[c]^T @ P[c]    (TensorE, PSUM-accumulated)
    """
    import concourse.mybir as mybir
    import concourse.tile as tile
    from concourse import bacc
    from concourse.bass import ds, ts

    f32 = mybir.dt.float32
    f16 = mybir.dt.float16
    nc = bacc.Bacc("TRN2", target_bir_lowering=False)

    FSUP_ = 1024            # free-dim super-chunk
    NSUP = FTOT // FSUP_    # 8
    NJ = FSUP_ // FCH       # 2 matmul FD chunks per super-chunk
    NEVAC = 3               # chunks whose G2 is evacuated by ScalarE

    x1gd = nc.dram_tensor("x1g", [nchunks, 128, FTOT], f16, kind="ExternalInput")
    x2d = nc.dram_tensor("x2f", [P_IN, FTOT], f16, kind="ExternalInput")
    s2d = nc.dram_tensor("sel2", [P_IN, nchunks * 128], f16, kind="ExternalInput")
    wd = nc.dram_tensor("wmat", [128, nchunks * P_IN], f16, kind="ExternalInput")
    outd = nc.dram_tensor("outf", [P_IN, FTOT], f16, kind="ExternalOutput")

    with tile.TileContext(nc) as tc:
        with tc.tile_pool(name="const", bufs=1) as constp, \
             tc.tile_pool(name="x1io", bufs=3 * nchunks) as x1io, \
             tc.tile_pool(name="x2io", bufs=3) as x2io, \
             tc.tile_pool(name="gsb", bufs=4) as gsb, \
             tc.tile_pool(name="psb", bufs=2 * nchunks) as psb, \
             tc.tile_pool(name="og", bufs=3) as og, \
             tc.tile_pool(name="psg", bufs=3, space="PSUM") as psg, \
             tc.tile_pool(name="pso", bufs=2, space="PSUM") as pso:

            s2 = constp.tile([P_IN, nchunks * 128], f16, tag="s2")
            nc.scalar.dma_start(out=s2, in_=s2d[:])
            w = constp.tile([128, nchunks * P_IN], f16, tag="w")
            nc.scalar.dma_start(out=w, in_=wd[:])

            for sup in range(NSUP):
                ssl = ds(sup * FSUP_, FSUP_)
                x2t = x2io.tile([P_IN, FSUP_], f16, tag="x2t")
                # SWDGE (GpSimd) queue: keeps ScalarE free for evacuations
                nc.gpsimd.dma_start(out=x2t, in_=x2d[:, ssl])
                x1gt = []
                for c in range(nchunks):
                    t = x1io.tile([128, FSUP_], f16, tag="x1g")
                    # the big streaming input gets its own (sync) HWDGE queue
                    nc.sync.dma_start(
                        out=t, in_=x1gd[c, :, sup * FSUP_:(sup + 1) * FSUP_])
                    x1gt.append(t)

                pts = []
                for c in range(nchunks):
                    g2p = psg.tile([128, FSUP_], f32, tag="gp")
                    for j in range(NJ):
                        nc.tensor.matmul(g2p[:, ts(j, FCH)], s2[:, ts(c, 128)],
                                         x2t[:, ts(j, FCH)], start=True, stop=True)
                    pt = psb.tile([128, FSUP_], f16, tag="pt")
                    if c < 2:
                        # PSUM-direct multiply on V (1x, but skips the evac)
                        nc.vector.tensor_mul(pt, x1gt[c], g2p)
                    else:
                        g2s = gsb.tile([128, FSUP_], f16, tag="g2s")
                        nc.scalar.copy(out=g2s, in_=g2p)
                        if c == 2:
                            # GpSimd offload; scatter consumes c2 mid-chain a
                            # full super later, so its ~2.3us latency hides
                            nc.gpsimd.tensor_mul(pt, x1gt[c], g2s)
                        else:
                            nc.vector.tensor_mul(pt, x1gt[c], g2s)
                    pts.append(pt)

                # scatter: W[c] PSUM-accumulated over c, one bank per j
                outps = []
                for j in range(NJ):
                    outp_j = pso.tile([P_IN, FCH], f32, tag="outp")
                    outps.append(outp_j)
                for c in range(nchunks):
                    for j in range(NJ):
                        nc.tensor.matmul(outps[j], w[:, ts(c, P_IN)],
                                         pts[c][:, ts(j, FCH)],
                                         start=(c == 0), stop=(c == nchunks - 1),
                                         skip_group_check=True)
                outt = og.tile([P_IN, FSUP_], f16, tag="outt")
                if sup == NSUP - 1:
                    # final super-chunk is the kernel tail: parallelize the two
                    # copies across V/S and ship via the low-latency HWDGE path
                    nc.vector.tensor_copy(out=outt[:, ts(0, FCH)], in_=outps[0])
                    nc.scalar.copy(out=outt[:, ts(1, FCH)], in_=outps[1])
                    nc.scalar.dma_start(out=outd[:, ssl], in_=outt)
                else:
                    for j in range(NJ):
                        nc.vector.tensor_copy(out=outt[:, ts(j, FCH)], in_=outps[j])
                    nc.gpsimd.dma_start(out=outd[:, ssl], in_=outt)
    nc.compile()
    return nc


def kernel(x1, x2, cg_tilde, repids_in1, repids_in2, repids_out, out_dim=DIM,
           **_ignored):
    global LAST_RESULTS
    import concourse.bass_utils as _bu
    from concourse.bass_utils import run_bass_kernel_spmd
    # the trace path uploads artifacts to S3, which this container can't reach
    if not getattr(_bu.upload_artifacts, "_local", False):
        _bu.upload_artifacts = lambda tmpdir: "local://" + tmpdir
        _bu.upload_artifacts._local = True

    x1 = np.ascontiguousarray(np.asarray(x1), dtype=np.float32)
    x2 = np.ascontiguousarray(np.asarray(x2), dtype=np.float32)
    cg = np.asarray(cg_tilde, dtype=np.float32)
    r1 = np.asarray(repids_in1, dtype=np.int64)
    r2 = np.asarray(repids_in2, dtype=np.int64)
    ro = np.asarray(repids_out, dtype=np.int64)
    out_dim = int(out_dim)
    assert x1.shape == (N, DIM) and x2.shape == (N, DIM) and out_dim == DIM

    A2, SEL2, WPACK, nchunks = _build_matrices(cg, r1, r2, ro)

    nc = _program_cache.get(nchunks)
    if nc is None:
        nc = _build_program(nchunks)
        _program_cache[nchunks] = nc

    in_maps = []
    for c in range(NCORES):
        sl = slice(c * NLOC, (c + 1) * NLOC)
        x1f = _pack_x(x1[sl])
        in_maps.append({
            "x1g": np.ascontiguousarray(
                x1f[A2].reshape(nchunks, 128, FTOT)),
            "x2f": _pack_x(x2[sl]),
            "sel2": SEL2,
            "wmat": WPACK,
        })

    res = run_bass_kernel_spmd(nc, in_maps, core_ids=list(range(NCORES)))
    LAST_RESULTS = res

    out = np.empty((N, DIM), np.float32)
    for c in range(NCORES):
        out[c * NLOC:(c + 1) * NLOC] = _unpack_out(
            np.asarray(res.results[c]["outf"], dtype=np.float32))
    return out


def _numpy_model(x1, x2, cg, r1, r2, ro):
    """Host-side model of the device dataflow (including fp16 quantization),
    for validating index logic and predicting the on-device error."""
    A2, SEL2, WPACK, nchunks = _build_matrices(cg, r1, r2, ro)
    W = np.zeros((128 * nchunks, P_IN), np.float32)
    for c in range(nchunks):
        W[c * 128:(c + 1) * 128, :] = WPACK[:, c * P_IN:(c + 1) * P_IN].astype(
            np.float32)
    out = np.empty_like(x1)
    for c in range(NCORES):
        sl = slice(c * NLOC, (c + 1) * NLOC)
        x1f = _pack_x(x1[sl])
        x2f = _pack_x(x2[sl]).astype(np.float32)
        g1 = x1f[A2].astype(np.float32)
        g2 = (SEL2.astype(np.float32).T @ x2f).astype(np.float16)  # worst branch
        p = (g1 * g2.astype(np.float32)).astype(np.float16)
        outf = W.T @ p.astype(np.float32)
        out[sl] = _unpack_out(outf)
    return out



# revision 19
# speedup vs baseline: 1.4608x; 1.1532x over previous
"""Trainium2 Bass kernel for nn_CGPCoupler (sparse Clebsch-Gordan bilinear coupling).

Reference computation:
    out[:, ro] += x1[:, r1] * x2[:, r2] * cg        (nnz = 9856 sparse entries)

Structure exploited: the index triples come in 16-wide aligned runs, so the whole
op factors over 16-element "subslots" (40 of them in the 640-dim rep space):

    out_O  +=  c_t * (x1_A  (*)  x2_B)      for 616 subslot-triples t=(A,B,O,c)

with only D=308 distinct (A,B) products. Dataflow (per core, data parallel over
the batch dim, 1024 rows/core, fp16 datapath / fp32 PSUM):

    layout:  x2f[p = subslot*2 + ch_half (80 partitions), f = n*8 + ch_lo (8192)]
    host:    x1g = x1 replicated into product-row order (numpy fancy-index),
             streamed straight from HBM (no on-chip gather for side 1)
    1. G2 = SEL2^T @ x2f      (TensorE one-hot selection matmul -> PSUM)
    2. P  = x1g * G2          (VectorE; 4 of 5 chunks evacuated to SBUF fp16 by
                               ScalarE first so the multiply runs in 2x mode)
    3. out = W^T @ P          (TensorE, CG coeffs folded into constant fp16 W,
                               PSUM-accumulated over the 5 product-row chunks)

Host-side numpy work (layout shuffles, building SEL2/W/x1g) is preprocessing of
inputs/constants; all arithmetic combining x1 and x2 happens on the NeuronCores.
"""

import os
import sys
import types

import numpy as np


def _ensure_ntff_hook():
    """concourse's trace path imports antenv.axon_hooks, which this image's
    antenv lacks. Provide it (and register the real profiling hook when the
    axon boot module is available) so tracing works instead of crashing."""
    try:
        import antenv
    except ImportError:
        return
    if getattr(antenv, "axon_hooks", None) is not None:
        return
    try:
        from antenv import axon_hooks  # noqa: F401
        return
    except ImportError:
        pass
    mod = types.ModuleType("antenv.axon_hooks")
    state = {"hook": None}
    mod.set_axon_ntff_profile_hook = lambda h: state.__setitem__("hook", h)
    mod.get_axon_ntff_profile_hook = lambda: state["hook"]
    sys.modules["antenv.axon_hooks"] = mod
    antenv.axon_hooks = mod
    try:
        from trn_agent_boot.trn_boot import _ntff_profile_via_ctypes
        so = "/opt/axon/libaxon_pjrt.so"
        if os.path.exists(so):
            mod.set_axon_ntff_profile_hook(_ntff_profile_via_ctypes(so))
    except Exception:
        pass


_ensure_ntff_hook()

N = 8192
DIM = 640
NCORES = 8
NLOC = N // NCORES          # rows per core
NSUB = DIM // 16            # 40 subslots
P_IN = NSUB * 2             # 80 partitions: (subslot, ch-half)
CHH = 8                     # channels per half
FTOT = NLOC * CHH           # 8192 free elements per partition
FSUP = 2048                 # free-dim super-chunk (per DMA / out tile)
FCH = 512                   # free-dim chunk per matmul (one PSUM bank, fp32)

LAST_RESULTS = None         # BassKernelResults of the most recent run

_matrices_cache = {}
_program_cache = {}


def _build_matrices(cg, r1, r2, ro):
    """Derive subslot terms from the sparse index lists and build the constant
    SEL1/SEL2/W matrices. Everything is validated with asserts."""
    key = (r1.tobytes(), r2.tobytes(), ro.tobytes(), cg.tobytes())
    hit = _matrices_cache.get(key)
    if hit is not None:
        return hit

    A = r1 // 16
    B = r2 // 16
    O = ro // 16
    j = r1 % 16
    assert (r2 % 16 == j).all() and (ro % 16 == j).all(), \
        "index triples are not 16-aligned runs"
    assert A.max() < NSUB and B.max() < NSUB and O.max() < NSUB

    terms = {}   # (A,B,O) -> [coeff, covered-bitmask]
    for a, b, o, jj, c in zip(A.tolist(), B.tolist(), O.tolist(),
                              j.tolist(), cg.tolist()):
        k = (a, b, o)
        e = terms.get(k)
        if e is None:
            terms[k] = [c, 1 << jj]
        else:
            assert e[0] == c, "coefficient varies within a 16-run"
            assert not (e[1] >> jj) & 1, "duplicate (A,B,O,j) entry"
            e[1] |= 1 << jj
    for k, (c, mask) in terms.items():
        assert mask == 0xFFFF, f"term {k} covers only mask {mask:#x}"

    products = sorted({(a, b) for (a, b, o) in terms})
    pidx = {ab: d for d, ab in enumerate(products)}
    D = len(products)
    D2 = 2 * D
    nchunks = (D2 + 127) // 128
    D2p = 128 * nchunks

    SEL2 = np.zeros((P_IN, D2p), np.float16)
    A2 = np.zeros(D2p, np.int64)   # product row -> source row in x1f layout
    W = np.zeros((D2p, P_IN), np.float16)
    for (a, b), d in pidx.items():
        for hh in (0, 1):
            SEL2[b * 2 + hh, 2 * d + hh] = 1.0
            A2[2 * d + hh] = a * 2 + hh
    for (a, b, o), (c, _) in terms.items():
        d = pidx[(a, b)]
        for hh in (0, 1):
            W[2 * d + hh, o * 2 + hh] = c

    # pack W row-chunks side by side: WPACK[:, c*P_IN:(c+1)*P_IN] = W[c*128:...]
    WPACK = np.zeros((128, nchunks * P_IN), np.float16)
    for c in range(nchunks):
        WPACK[:, c * P_IN:(c + 1) * P_IN] = W[c * 128:(c + 1) * 128, :]

    out = (A2, SEL2, WPACK, nchunks)
    _matrices_cache[key] = out
    return out


def _pack_x(x):
    """[NLOC, 640] -> [80, NLOC*8] fp16: row p = subslot*2 + half, col = n*8 + ch."""
    return np.ascontiguousarray(
        x.reshape(NLOC, NSUB, 2, CHH).transpose(1, 2, 0, 3).reshape(P_IN, FTOT),
        dtype=np.float16)


def _unpack_out(o):
    """[80, NLOC*8] -> [NLOC, 640]."""
    return o.reshape(NSUB, 2, NLOC, CHH).transpose(2, 0, 1, 3).reshape(NLOC, DIM)


def _build_program(nchunks):
    """fp16 datapath, v3: the G1 side (x1 replicated into product-row order) is
    prepared on the host and streamed straight from HBM — no gather matmul and
    no PSUM round-trip for it. On-chip work per super-chunk of 1024 free elems:
      - G2 = SEL2^T @ x2f  (TensorE -> PSUM)
      - P[c] = x1g[c] * G2[c]   (VectorE; for NEVAC chunks ScalarE first
        evacuates G2 to SBUF fp16 so the multiply runs in 2x 16-bit mode)
      - out += W[c]^T @ P[c]    (TensorE, PSUM-accumulated)
    """
    import concourse.mybir as mybir
    import concourse.tile as tile
    from concourse import bacc
    from concourse.bass import ds, ts

    f32 = mybir.dt.float32
    f16 = mybir.dt.float16
    nc = bacc.Bacc("TRN2", target_bir_lowering=False)

    FSUP_ = 1024            # free-dim super-chunk
    NSUP = FTOT // FSUP_    # 8
    NJ = FSUP_ // FCH       # 2 matmul FD chunks per super-chunk
    NEVAC = 4               # chunks whose G2 is evacuated by ScalarE (2x TT on V)

    x1gd = nc.dram_tensor("x1g", [nchunks, 128, FTOT], f16, kind="ExternalInput")
    x2d = nc.dram_tensor("x2f", [P_IN, FTOT], f16, kind="ExternalInput")
    s2d = nc.dram_tensor("sel2", [P_IN, nchunks * 128], f16, kind="ExternalInput")
    wd = nc.dram_tensor("wmat", [128, nchunks * P_IN], f16, kind="ExternalInput")
    outd = nc.dram_tensor("outf", [P_IN, FTOT], f16, kind="ExternalOutput")

    with tile.TileContext(nc) as tc:
        with tc.tile_pool(name="const", bufs=1) as constp, \
             tc.tile_pool(name="x1io", bufs=3 * nchunks) as x1io, \
             tc.tile_pool(name="x2io", bufs=3) as x2io, \
             tc.tile_pool(name="gsb", bufs=4) as gsb, \
             tc.tile_pool(name="psb", bufs=2 * nchunks) as psb, \
             tc.tile_pool(name="og", bufs=3) as og, \
             tc.tile_pool(name="psg", bufs=3, space="PSUM") as psg, \
             tc.tile_pool(name="pso", bufs=2, space="PSUM") as pso:

            s2 = constp.tile([P_IN, nchunks * 128], f16, tag="s2")
            nc.scalar.dma_start(out=s2, in_=s2d[:])
            w = constp.tile([128, nchunks * P_IN], f16, tag="w")
            nc.scalar.dma_start(out=w, in_=wd[:])

            for sup in range(NSUP):
                ssl = ds(sup * FSUP_, FSUP_)
                x2t = x2io.tile([P_IN, FSUP_], f16, tag="x2t")
                # SWDGE (GpSimd) queue: keeps ScalarE free for evacuations
                nc.gpsimd.dma_start(out=x2t, in_=x2d[:, ssl])
                x1gt = []
                for c in range(nchunks):
                    t = x1io.tile([128, FSUP_], f16, tag="x1g")
                    # the big streaming input gets its own (sync) HWDGE queue
                    nc.sync.dma_start(
                        out=t, in_=x1gd[c, :, sup * FSUP_:(sup + 1) * FSUP_])
                    x1gt.append(t)

                pts = []
                for c in range(nchunks):
                    g2p = psg.tile([128, FSUP_], f32, tag="gp")
                    for j in range(NJ):
                        nc.tensor.matmul(g2p[:, ts(j, FCH)], s2[:, ts(c, 128)],
                                         x2t[:, ts(j, FCH)], start=True, stop=True)
                    pt = psb.tile([128, FSUP_], f16, tag="pt")
                    if c >= nchunks - NEVAC:
                        # evacuated chunks: TT runs in 2x 16-bit mode; kept last
                        # so the scatter isn't gated by the slow psum-read TT
                        # (chunk 0's 1x TT hides under the remaining gathers)
                        g2s = gsb.tile([128, FSUP_], f16, tag="g2s")
                        nc.scalar.copy(out=g2s, in_=g2p)
                        nc.vector.tensor_mul(pt, x1gt[c], g2s)
                    else:
                        nc.vector.tensor_mul(pt, x1gt[c], g2p)
                    pts.append(pt)

                # scatter: W[c] PSUM-accumulated over c, one bank per j
                outps = []
                for j in range(NJ):
                    outp_j = pso.tile([P_IN, FCH], f32, tag="outp")
                    outps.append(outp_j)
                for c in range(nchunks):
                    for j in range(NJ):
                        nc.tensor.matmul(outps[j], w[:, ts(c, P_IN)],
                                         pts[c][:, ts(j, FCH)],
                                         start=(c == 0), stop=(c == nchunks - 1),
                                         skip_group_check=True)
                outt = og.tile([P_IN, FSUP_], f16, tag="outt")
                if sup == NSUP - 1:
                    # final super-chunk is the kernel tail: parallelize the two
                    # copies across V/S and ship via the low-latency HWDGE path
                    nc.vector.tensor_copy(out=outt[:, ts(0, FCH)], in_=outps[0])
                    nc.scalar.copy(out=outt[:, ts(1, FCH)], in_=outps[1])
                    nc.scalar.dma_start(out=outd[:, ssl], in_=outt)
                else:
                    for j in range(NJ):
                        nc.vector.tensor_copy(out=outt[:, ts(j, FCH)], in_=outps[j])
                    nc.gpsimd.dma_start(out=outd[:, ssl], in_=outt)
    nc.compile()
    return nc


def kernel(x1, x2, cg_tilde, repids_in1, repids_in2, repids_out, out_dim=DIM,
           **_ignored):
    global LAST_RESULTS
    import concourse.bass_utils as _bu
    from concourse.bass_utils import run_bass_kernel_spmd
    # the trace path uploads artifacts to S3, which this container can't reach
    if not getattr(_bu.upload_artifacts, "_local", False):
        _bu.upload_artifacts = lambda tmpdir: "local://" + tmpdir
        _bu.upload_artifacts._local = True

    x1 = np.ascontiguousarray(np.asarray(x1), dtype=np.float32)
    x2 = np.ascontiguousarray(np.asarray(x2), dtype=np.float32)
    cg = np.asarray(cg_tilde, dtype=np.float32)
    r1 = np.asarray(repids_in1, dtype=np.int64)
    r2 = np.asarray(repids_in2, dtype=np.int64)
    ro = np.asarray(repids_out, dtype=np.int64)
    out_dim = int(out_dim)
    assert x1.shape == (N, DIM) and x2.shape == (N, DIM) and out_dim == DIM

    A2, SEL2, WPACK, nchunks = _build_matrices(cg, r1, r2, ro)

    nc = _program_cache.get(nchunks)
    if nc is None:
        nc = _build_program(nchunks)
        _program_cache[nchunks] = nc

    in_maps = []
    for c in range(NCORES):
        sl = slice(c * NLOC, (c + 1) * NLOC)
        x1f = _pack_x(x1[sl])
        in_maps.append({
            "x1g": np.ascontiguousarray(
                x1f[A2].reshape(nchunks, 128, FTOT)),
            "x2f": _pack_x(x2[sl]),
            "sel2": SEL2,
            "wmat": WPACK,
        })

    res = run_bass_kernel_spmd(nc, in_maps, core_ids=list(range(NCORES)))
    LAST_RESULTS = res

    out = np.empty((N, DIM), np.float32)
    for c in range(NCORES):
        out[c * NLOC:(c + 1) * NLOC] = _unpack_out(
            np.asarray(res.results[c]["outf"], dtype=np.float32))
    return out


def _numpy_model(x1, x2, cg, r1, r2, ro):
    """Host-side model of the device dataflow (including fp16 quantization),
    for validating index logic and predicting the on-device error."""
    A2, SEL2, WPACK, nchunks = _build_matrices(cg, r1, r2, ro)
    W = np.zeros((128 * nchunks, P_IN), np.float32)
    for c in range(nchunks):
        W[c * 128:(c + 1) * 128, :] = WPACK[:, c * P_IN:(c + 1) * P_IN].astype(
            np.float32)
    out = np.empty_like(x1)
    for c in range(NCORES):
        sl = slice(c * NLOC, (c + 1) * NLOC)
        x1f = _pack_x(x1[sl])
        x2f = _pack_x(x2[sl]).astype(np.float32)
        g1 = x1f[A2].astype(np.float32)
        g2 = (SEL2.astype(np.float32).T @ x2f).astype(np.float16)  # worst branch
        p = (g1 * g2.astype(np.float32)).astype(np.float16)
        outf = W.T @ p.astype(np.float32)
        out[sl] = _unpack_out(outf)
    return out

